# revision 17
# baseline (speedup 1.0000x reference)
"""Multi-head linear attention (elu+1 feature map) on 8 TRN2 NeuronCores.

Sharding: core c handles batch b = c//2, sequence half j = c%2 (2048 rows).
Each core computes q/k/v projections + phi + partial kv/z for its rows,
AllReduces kv/z across the (b, j) pair, then computes num/den/ctx and the
output projection for its rows. All matmuls in bf16 (fp32 PSUM accumulate).

Device-side layout notes:
  - query slice is DMA-transposed (bf16 xbar) into feature-major xT
    pair-blocks (128 = 2 heads x 64 dims, S) so projections contract the
    feature dim on partitions with block-diagonal weights.
  - kf/v are produced sequence-major (s on partitions) by using xT chunks
    as the matmul stationary; kv/z accumulate in PSUM over s-chunks.
  - phi(x) = elu(x)+1 is computed exactly as max(x+1, exp(min(x+1,1)-1)).
  - num/den use feature-major qfT; den rows for all 16 heads accumulate in
    one PSUM tile via zero-padded z columns; division by den is fused into
    the num PSUM eviction against a partition-broadcast reciprocal.
  - output projection consumes feature-major ctxT as stationary and natural
    Wo row-blocks as moving operand, yielding sequence-major output.
"""
import numpy as np
import ml_dtypes

B, S, H, Dh = 4, 4096, 16, 64
E = H * Dh
N_CORES = 8
SL = S // 2          # sequence rows per core
NPAIR = H // 2       # head pairs
EPS = 1e-6

_CACHE = {}


def _build_program():
    import concourse.bacc as bacc
    import concourse.mybir as mybir
    import concourse.tile as tile

    bf16 = mybir.dt.bfloat16
    f32 = mybir.dt.float32
    Act = mybir.ActivationFunctionType
    Alu = mybir.AluOpType

    nc = bacc.Bacc(None, target_bir_lowering=False, num_devices=N_CORES)

    xq = nc.dram_tensor("xq", [SL, E], bf16, kind="ExternalInput")
    wq_bd = nc.dram_tensor("wq_bd", [NPAIR, 128, 128], bf16, kind="ExternalInput")
    wkv_bd = nc.dram_tensor("wkv_bd", [NPAIR, 128, 256], bf16, kind="ExternalInput")
    wo = nc.dram_tensor("wo", [E, E], bf16, kind="ExternalInput")
    y = nc.dram_tensor("y", [SL, E], f32, kind="ExternalOutput")
    kv_ar = nc.dram_tensor("kv_ar", [128, NPAIR * 129], f32)

    NCHUNK = SL // 128   # s-chunks per pair for kf/v (16)
    NQC = SL // 512      # 512-wide chunks for qf / num / den (4)
    NSO = SL // 128      # output row chunks (16)

    with tile.TileContext(nc) as tc:
        with (
            tc.tile_pool(name="persist", bufs=1) as persist,
            tc.tile_pool(name="xp", bufs=2) as xp,
            tc.tile_pool(name="kvsb", bufs=2) as kvsb,
            tc.tile_pool(name="tmp", bufs=3) as tmp,
            tc.tile_pool(name="outp", bufs=3) as outp,
            tc.tile_pool(name="dram", bufs=1, space="DRAM") as dram,
        ):
            # ---- weights / constants ----
            wq_sb = persist.tile([128, NPAIR, 128], bf16)
            nc.sync.dma_start(out=wq_sb[:], in_=wq_bd.rearrange("p k m -> k p m"))
            wkv_sb = persist.tile([128, NPAIR, 256], bf16)
            nc.sync.dma_start(out=wkv_sb[:], in_=wkv_bd.rearrange("p k m -> k p m"))
            wo_sb = persist.tile([128, NPAIR, E], bf16)
            nc.sync.dma_start(out=wo_sb[:], in_=wo.rearrange("(k p) n -> p k n", p=128))
            qfT = persist.tile([128, NPAIR, SL], bf16)
            ctxT = persist.tile([128, NPAIR, SL], bf16)
            kvstage = persist.tile([128, NPAIR, 129], f32)

            # ---- phase K: per pair, kf/v (s-major) + kv/z + qf (feature-major)
            with (
                tc.tile_pool(name="ps_kvp", bufs=2, space="PSUM") as ps_kvp,
                tc.tile_pool(name="ps_kv", bufs=2, space="PSUM") as ps_kv,
                tc.tile_pool(name="ps_q", bufs=2, space="PSUM") as ps_q,
            ):
                for p in range(NPAIR):
                    xT = xp.tile([128, SL], bf16, tag="xT")
                    nc.sync.dma_start(
                        out=xT[:], in_=xq[:, p * 128:(p + 1) * 128], transpose=True
                    )

                    kf = kvsb.tile([128, NCHUNK, 128], bf16, tag="kf")
                    vsb = kvsb.tile([128, NCHUNK, 130], bf16, tag="v")
                    nc.vector.memset(vsb[:, :, 128:129], 1.0)
                    kvacc = ps_kv.tile([128, 130], f32, tag="kvacc")
                    for g in range(NCHUNK // 4):
                        kvps = ps_kvp.tile([128, 1024], f32, tag="kvps")
                        kvps3 = kvps[:].rearrange("q (c x) -> q c x", c=4)
                        for c4 in range(4):
                            i = g * 4 + c4
                            nc.tensor.matmul(
                                kvps3[:, c4, :],
                                lhsT=xT[:, i * 128:(i + 1) * 128],
                                rhs=wkv_sb[:, p, :],
                                start=True, stop=True,
                            )
                        cs = slice(g * 4, (g + 1) * 4)
                        kA = tmp.tile([128, 4, 128], bf16, tag="kA")
                        nc.vector.tensor_scalar_add(kA[:], kvps3[:, :, 0:128], 1.0)
                        nc.vector.tensor_copy(vsb[:, cs, 0:128], kvps3[:, :, 128:256])
                        kM = tmp.tile([128, 4, 128], bf16, tag="kM")
                        nc.vector.tensor_scalar(
                            kM[:], kA[:], 1.0, -1.0, Alu.min, Alu.add
                        )
                        kE = tmp.tile([128, 4, 128], bf16, tag="kE")
                        nc.scalar.activation(kE[:], kM[:], Act.Exp)
                        nc.vector.tensor_tensor(kf[:, cs, :], kA[:], kE[:], Alu.max)
                        for c4 in range(4):
                            i = g * 4 + c4
                            nc.tensor.matmul(
                                kvacc[:, 0:129],
                                lhsT=kf[:, i, :], rhs=vsb[:, i, 0:129],
                                start=(i == 0), stop=(i == NCHUNK - 1),
                            )
                    nc.vector.tensor_copy(kvstage[:, p, 0:129], kvacc[:, 0:129])

                    for qc in range(NQC):
                        qs = slice(qc * 512, (qc + 1) * 512)
                        qps = ps_q.tile([128, 512], f32, tag="qps")
                        nc.tensor.matmul(
                            qps[:], lhsT=wq_sb[:, p, :], rhs=xT[:, qs],
                            start=True, stop=True,
                        )
                        qA = tmp.tile([128, 512], bf16, tag="qA")
                        nc.vector.tensor_scalar_add(qA[:], qps[:], 1.0)
                        qM = tmp.tile([128, 512], bf16, tag="qM")
                        nc.vector.tensor_scalar(
                            qM[:], qA[:], 1.0, -1.0, Alu.min, Alu.add
                        )
                        qE = tmp.tile([128, 512], bf16, tag="qE")
                        nc.scalar.activation(qE[:], qM[:], Act.Exp)
                        nc.vector.tensor_tensor(qfT[:, p, qs], qA[:], qE[:], Alu.max)

            # ---- phase R: AllReduce kv/z across the batch pair ----
            kv_in = dram.tile([128, NPAIR * 129], f32)
            nc.sync.dma_start(
                out=kv_in[:], in_=kvstage[:].rearrange("q p c -> q (p c)")
            )
            nc.gpsimd.collective_compute(
                "AllReduce",
                Alu.add,
                replica_groups=[[0, 1], [2, 3], [4, 5], [6, 7]],
                ins=[kv_in[:]],
                outs=[kv_ar[:]],
            )
            kvrd = persist.tile([128, NPAIR, 129], f32)
            nc.sync.dma_start(
                out=kvrd[:], in_=kv_ar.rearrange("q (p c) -> q p c", c=129)
            )

            # ---- phase N1: den for all heads -> reciprocal ----
            # zbd is per-pair: only pair p's two head columns are nonzero, so
            # the accumulated matmuls sum z_h . qf_h over the right pair.
            zbd = persist.tile([128, NPAIR, H], bf16)
            nc.vector.memset(zbd[:], 0.0)
            for p in range(NPAIR):
                nc.vector.tensor_copy(
                    zbd[0:64, p, 2 * p:2 * p + 1], kvrd[0:64, p, 128:129]
                )
                nc.vector.tensor_copy(
                    zbd[64:128, p, 2 * p + 1:2 * p + 2], kvrd[64:128, p, 128:129]
                )
            with tc.tile_pool(name="ps_den", bufs=1, space="PSUM") as ps_den:
                denps = ps_den.tile([16, SL], f32)
                for p in range(NPAIR):
                    for qc in range(NQC):
                        qs = slice(qc * 512, (qc + 1) * 512)
                        nc.tensor.matmul(
                            denps[:, qs], lhsT=zbd[:, p, :], rhs=qfT[:, p, qs],
                            start=(p == 0), stop=(p == NPAIR - 1),
                        )
                den_sb = persist.tile([16, SL], f32)
                nc.vector.tensor_scalar_add(den_sb[:], denps[:], EPS)
            recip = persist.tile([16, SL], f32)
            nc.vector.reciprocal(recip[:], den_sb[:])
            recip_bf = persist.tile([16, SL], bf16)
            nc.vector.tensor_copy(recip_bf[:], recip[:])
            recip_dram = dram.tile([16, SL], bf16)
            nc.sync.dma_start(out=recip_dram[:], in_=recip_bf[:])

            # ---- phase N2: num + fused divide -> ctxT ----
            with tc.tile_pool(name="ps_num", bufs=3, space="PSUM") as ps_num:
                for p in range(NPAIR):
                    kvbd = tmp.tile([128, 128], bf16, tag="kvbd")
                    nc.vector.memset(kvbd[:], 0.0)
                    nc.vector.tensor_copy(kvbd[0:64, 0:64], kvrd[0:64, p, 0:64])
                    nc.vector.tensor_copy(
                        kvbd[64:128, 64:128], kvrd[64:128, p, 64:128]
                    )
                    rbc = tmp.tile([128, SL], bf16, tag="rbc")
                    nc.sync.dma_start(
                        out=rbc[0:64, :],
                        in_=recip_dram[2 * p:2 * p + 1, :].to_broadcast([64, SL]),
                    )
                    nc.sync.dma_start(
                        out=rbc[64:128, :],
                        in_=recip_dram[2 * p + 1:2 * p + 2, :].to_broadcast([64, SL]),
                    )
                    for qc in range(NQC):
                        qs = slice(qc * 512, (qc + 1) * 512)
                        nps = ps_num.tile([128, 512], f32, tag="nps")
                        nc.tensor.matmul(
                            nps[:], lhsT=kvbd[:], rhs=qfT[:, p, qs],
                            start=True, stop=True,
                        )
                        nc.vector.tensor_tensor(
                            ctxT[:, p, qs], nps[:], rbc[:, qs], Alu.mult
                        )

            # ---- phase O: output projection (sequence-major out) ----
            with tc.tile_pool(name="ps_o", bufs=2, space="PSUM") as ps_o:
                for si in range(NSO):
                    ss = slice(si * 128, (si + 1) * 128)
                    ops = ps_o.tile([128, E], f32, tag="ops")
                    for k in range(NPAIR):
                        nc.tensor.matmul(
                            ops[:, 0:512], lhsT=ctxT[:, k, ss],
                            rhs=wo_sb[:, k, 0:512],
                            start=(k == 0), stop=(k == NPAIR - 1),
                        )
                        nc.tensor.matmul(
                            ops[:, 512:E], lhsT=ctxT[:, k, ss],
                            rhs=wo_sb[:, k, 512:E],
                            start=(k == 0), stop=(k == NPAIR - 1),
                        )
                    ysb = outp.tile([128, E], f32, tag="ysb")
                    nc.vector.tensor_copy(ysb[:], ops[:])
                    nc.sync.dma_start(out=y[ss, :], in_=ysb[:])

    nc.compile()
    return nc


def _get_program():
    if "nc" not in _CACHE:
        _CACHE["nc"] = _build_program()
    return _CACHE["nc"]


def _host_prep(query, Wq, Wk, Wv, Wo):
    bf16 = ml_dtypes.bfloat16
    q_bf = np.ascontiguousarray(query.astype(bf16))
    wq_bd = np.zeros((NPAIR, 128, 128), dtype=bf16)
    wkv_bd = np.zeros((NPAIR, 128, 256), dtype=bf16)
    for p in range(NPAIR):
        wq_bd[p, 0:64, 0:64] = Wq[2 * p]
        wq_bd[p, 64:128, 64:128] = Wq[2 * p + 1]
        wkv_bd[p, 0:64, 0:64] = Wk[2 * p]
        wkv_bd[p, 64:128, 64:128] = Wk[2 * p + 1]
        wkv_bd[p, 0:64, 128:192] = Wv[2 * p]
        wkv_bd[p, 64:128, 192:256] = Wv[2 * p + 1]
    wo_bf = np.ascontiguousarray(Wo.astype(bf16))
    in_maps = []
    for c in range(N_CORES):
        b, j = divmod(c, 2)
        in_maps.append({
            "xq": np.ascontiguousarray(q_bf[b, j * SL:(j + 1) * SL, :]),
            "wq_bd": wq_bd,
            "wkv_bd": wkv_bd,
            "wo": wo_bf,
        })
    return in_maps


def kernel(query, Wq, Wk, Wv, Wo):
    from concourse.bass_utils import run_bass_kernel_spmd

    nc = _get_program()
    in_maps = _host_prep(query, Wq, Wk, Wv, Wo)
    res = run_bass_kernel_spmd(nc, in_maps, list(range(N_CORES)))
    out = np.empty((B, S, E), dtype=np.float32)
    for c in range(N_CORES):
        b, j = divmod(c, 2)
        out[b, j * SL:(j + 1) * SL, :] = res.results[c]["y"]
    return out


# revision 22
# speedup vs baseline: 1.0560x; 1.0560x over previous
"""Multi-head linear attention (elu+1 feature map) on 8 TRN2 NeuronCores.

Sharding: core c handles batch b = c//2, sequence half j = c%2 (2048 rows).
Each core computes q/k/v projections + phi + partial kv/z for its rows,
AllReduces kv/z across the (b, j) pair, then computes num/den/ctx and the
output projection for its rows. All matmuls in bf16 (fp32 PSUM accumulate).

Device-side layout notes:
  - query slice is DMA-transposed (bf16 xbar) into feature-major xT
    pair-blocks (128 = 2 heads x 64 dims, S) so projections contract the
    feature dim on partitions with block-diagonal weights.
  - kf/v are produced sequence-major (s on partitions) by using xT chunks
    as the matmul stationary; kv/z accumulate in PSUM over s-chunks.
  - phi(x) = elu(x)+1 is computed exactly as max(x+1, exp(min(x+1,1)-1)).
  - num/den use feature-major qfT; den rows for all 16 heads accumulate in
    one PSUM tile via zero-padded z columns; division by den is fused into
    the num PSUM eviction against a partition-broadcast reciprocal.
  - output projection consumes feature-major ctxT as stationary and natural
    Wo row-blocks as moving operand, yielding sequence-major output.
"""
import numpy as np
import ml_dtypes

B, S, H, Dh = 4, 4096, 16, 64
E = H * Dh
N_CORES = 8
SL = S // 2          # sequence rows per core
NPAIR = H // 2       # head pairs
EPS = 1e-6

_CACHE = {}


def _build_program():
    import concourse.bacc as bacc
    import concourse.mybir as mybir
    import concourse.tile as tile

    bf16 = mybir.dt.bfloat16
    f32 = mybir.dt.float32
    Act = mybir.ActivationFunctionType
    Alu = mybir.AluOpType

    nc = bacc.Bacc(None, target_bir_lowering=False, num_devices=N_CORES)

    xq = nc.dram_tensor("xq", [SL, E], bf16, kind="ExternalInput")
    wq_bd = nc.dram_tensor("wq_bd", [NPAIR, 128, 128], bf16, kind="ExternalInput")
    wkv_bd = nc.dram_tensor("wkv_bd", [NPAIR, 128, 256], bf16, kind="ExternalInput")
    wo = nc.dram_tensor("wo", [E, E], bf16, kind="ExternalInput")
    y = nc.dram_tensor("y", [SL, E], f32, kind="ExternalOutput")
    kv_ar = nc.dram_tensor("kv_ar", [128, NPAIR * 129], f32)

    NCHUNK = SL // 128   # s-chunks per pair for kf/v (16)
    NQC = SL // 512      # 512-wide chunks for qf / num / den (4)
    NSO = SL // 128      # output row chunks (16)

    with tile.TileContext(nc) as tc:
        with (
            tc.tile_pool(name="persist", bufs=1) as persist,
            tc.tile_pool(name="xp", bufs=2) as xp,
            tc.tile_pool(name="kvsb", bufs=2) as kvsb,
            tc.tile_pool(name="tmp", bufs=2) as tmp,
            tc.tile_pool(name="rbcp", bufs=2) as rbcp,
            tc.tile_pool(name="outp", bufs=2) as outp,
            tc.tile_pool(name="dram", bufs=1, space="DRAM") as dram,
        ):
            # ---- weights / constants ----
            wq_sb = persist.tile([128, NPAIR, 128], bf16)
            nc.sync.dma_start(out=wq_sb[:], in_=wq_bd.rearrange("p k m -> k p m"))
            wkv_sb = persist.tile([128, NPAIR, 256], bf16)
            nc.sync.dma_start(out=wkv_sb[:], in_=wkv_bd.rearrange("p k m -> k p m"))
            wo_sb = persist.tile([128, NPAIR, E], bf16)
            nc.sync.dma_start(out=wo_sb[:], in_=wo.rearrange("(k p) n -> p k n", p=128))
            qfT = persist.tile([128, NPAIR, SL], bf16)
            ctxT = persist.tile([128, NPAIR, SL], bf16)

            # All 8 xT pair blocks stay resident (32 KiB/partition total).
            xTs = []
            for p in range(NPAIR):
                xT = xp.tile([128, SL], bf16, tag=f"xT{p}")
                nc.sync.dma_start(
                    out=xT[:], in_=xq[:, p * 128:(p + 1) * 128], transpose=True
                )
                xTs.append(xT)

            # ---- phase K1: kf/v (s-major) + kv/z for every pair ----
            kv_in = dram.tile([128, NPAIR * 129], f32)
            with (
                tc.tile_pool(name="ps_kvp", bufs=2, space="PSUM") as ps_kvp,
                tc.tile_pool(name="ps_kv", bufs=2, space="PSUM") as ps_kv,
            ):
                for p in range(NPAIR):
                    xT = xTs[p]
                    kf = kvsb.tile([128, NCHUNK, 128], bf16, tag="kf")
                    vsb = kvsb.tile([128, NCHUNK, 130], bf16, tag="v")
                    nc.vector.memset(vsb[:, :, 128:129], 1.0)
                    kvacc = ps_kv.tile([128, 130], f32, tag="kvacc")
                    for g in range(NCHUNK // 4):
                        kvps = ps_kvp.tile([128, 1024], f32, tag="kvps")
                        kvps3 = kvps[:].rearrange("q (c x) -> q c x", c=4)
                        for c4 in range(4):
                            i = g * 4 + c4
                            nc.tensor.matmul(
                                kvps3[:, c4, :],
                                lhsT=xT[:, i * 128:(i + 1) * 128],
                                rhs=wkv_sb[:, p, :],
                                start=True, stop=True,
                            )
                        cs = slice(g * 4, (g + 1) * 4)
                        kA = tmp.tile([128, 4, 128], bf16, tag="kA")
                        nc.vector.tensor_scalar_add(kA[:], kvps3[:, :, 0:128], 1.0)
                        nc.vector.tensor_copy(vsb[:, cs, 0:128], kvps3[:, :, 128:256])
                        kM = tmp.tile([128, 4, 128], bf16, tag="kM")
                        nc.vector.tensor_scalar(
                            kM[:], kA[:], 1.0, -1.0, Alu.min, Alu.add
                        )
                        nc.scalar.activation(kM[:], kM[:], Act.Exp)
                        nc.vector.tensor_tensor(kf[:, cs, :], kA[:], kM[:], Alu.max)
                        for c4 in range(4):
                            i = g * 4 + c4
                            nc.tensor.matmul(
                                kvacc[:, 0:129],
                                lhsT=kf[:, i, :], rhs=vsb[:, i, 0:129],
                                start=(i == 0), stop=(i == NCHUNK - 1),
                            )
                    kvst = outp.tile([128, 129], f32, tag="kvst")
                    nc.vector.tensor_copy(kvst[:], kvacc[:, 0:129])
                    nc.sync.dma_start(
                        out=kv_in[:, p * 129:(p + 1) * 129], in_=kvst[:]
                    )

            # ---- phase R: AllReduce kv/z (overlaps phase K2 below) ----
            nc.gpsimd.collective_compute(
                "AllReduce",
                Alu.add,
                replica_groups=[[0, 1], [2, 3], [4, 5], [6, 7]],
                ins=[kv_in[:]],
                outs=[kv_ar[:]],
            )

            # ---- phase K2: qf (feature-major), overlapping the collective ----
            with tc.tile_pool(name="ps_q", bufs=3, space="PSUM") as ps_q:
                for p in range(NPAIR):
                    xT = xTs[p]
                    for qc in range(2):
                        qs = slice(qc * 1024, (qc + 1) * 1024)
                        qps = ps_q.tile([128, 1024], f32, tag="qps")
                        nc.tensor.matmul(
                            qps[:, 0:512], lhsT=wq_sb[:, p, :],
                            rhs=xT[:, qc * 1024:qc * 1024 + 512],
                            start=True, stop=True,
                        )
                        nc.tensor.matmul(
                            qps[:, 512:1024], lhsT=wq_sb[:, p, :],
                            rhs=xT[:, qc * 1024 + 512:(qc + 1) * 1024],
                            start=True, stop=True,
                        )
                        qA = tmp.tile([128, 1024], bf16, tag="qA")
                        nc.scalar.activation(qA[:], qps[:], Act.Identity, bias=1.0)
                        qM = tmp.tile([128, 1024], bf16, tag="qM")
                        nc.vector.tensor_scalar(
                            qM[:], qA[:], 1.0, -1.0, Alu.min, Alu.add
                        )
                        nc.scalar.activation(qM[:], qM[:], Act.Exp)
                        nc.vector.tensor_tensor(qfT[:, p, qs], qA[:], qM[:], Alu.max)
            kvrd = persist.tile([128, NPAIR, 129], f32)
            nc.sync.dma_start(
                out=kvrd[:], in_=kv_ar.rearrange("q (p c) -> q p c", c=129)
            )

            # ---- phase N1: den for all heads -> reciprocal ----
            # zbd is per-pair: only pair p's two head columns are nonzero, so
            # the accumulated matmuls sum z_h . qf_h over the right pair.
            zbd = persist.tile([128, NPAIR, H], bf16)
            nc.vector.memset(zbd[:], 0.0)
            for p in range(NPAIR):
                nc.vector.tensor_copy(
                    zbd[0:64, p, 2 * p:2 * p + 1], kvrd[0:64, p, 128:129]
                )
                nc.vector.tensor_copy(
                    zbd[64:128, p, 2 * p + 1:2 * p + 2], kvrd[64:128, p, 128:129]
                )
            with tc.tile_pool(name="ps_den", bufs=1, space="PSUM") as ps_den:
                denps = ps_den.tile([16, SL], f32)
                for p in range(NPAIR):
                    for qc in range(NQC):
                        qs = slice(qc * 512, (qc + 1) * 512)
                        nc.tensor.matmul(
                            denps[:, qs], lhsT=zbd[:, p, :], rhs=qfT[:, p, qs],
                            start=(p == 0), stop=(p == NPAIR - 1),
                        )
                den_sb = persist.tile([16, SL], f32)
                nc.vector.tensor_scalar_add(den_sb[:], denps[:], EPS)
            nc.vector.reciprocal(den_sb[:], den_sb[:])
            recip_bf = persist.tile([16, SL], bf16)
            nc.vector.tensor_copy(recip_bf[:], den_sb[:])
            recip_dram = dram.tile([16, SL], bf16)
            nc.sync.dma_start(out=recip_dram[:], in_=recip_bf[:])

            # ---- phase N2: num + fused divide -> ctxT ----
            with tc.tile_pool(name="ps_num", bufs=3, space="PSUM") as ps_num:
                for p in range(NPAIR):
                    kvbd = tmp.tile([128, 128], bf16, tag="kvbd")
                    nc.vector.memset(kvbd[:], 0.0)
                    nc.vector.tensor_copy(kvbd[0:64, 0:64], kvrd[0:64, p, 0:64])
                    nc.vector.tensor_copy(
                        kvbd[64:128, 64:128], kvrd[64:128, p, 64:128]
                    )
                    for qc in range(2):
                        qs = slice(qc * 1024, (qc + 1) * 1024)
                        rbc = rbcp.tile([128, 1024], bf16, tag="rbc")
                        nc.sync.dma_start(
                            out=rbc[0:64, :],
                            in_=recip_dram[2 * p:2 * p + 1, qs].to_broadcast(
                                [64, 1024]
                            ),
                        )
                        nc.sync.dma_start(
                            out=rbc[64:128, :],
                            in_=recip_dram[2 * p + 1:2 * p + 2, qs].to_broadcast(
                                [64, 1024]
                            ),
                        )
                        nps = ps_num.tile([128, 1024], f32, tag="nps")
                        nc.tensor.matmul(
                            nps[:, 0:512], lhsT=kvbd[:],
                            rhs=qfT[:, p, qc * 1024:qc * 1024 + 512],
                            start=True, stop=True,
                        )
                        nc.tensor.matmul(
                            nps[:, 512:1024], lhsT=kvbd[:],
                            rhs=qfT[:, p, qc * 1024 + 512:(qc + 1) * 1024],
                            start=True, stop=True,
                        )
                        nc.vector.tensor_tensor(
                            ctxT[:, p, qs], nps[:], rbc[:], Alu.mult
                        )

            # ---- phase O: output projection (sequence-major out) ----
            with tc.tile_pool(name="ps_o", bufs=2, space="PSUM") as ps_o:
                for si in range(NSO):
                    ss = slice(si * 128, (si + 1) * 128)
                    ops = ps_o.tile([128, E], f32, tag="ops")
                    for k in range(NPAIR):
                        nc.tensor.matmul(
                            ops[:, 0:512], lhsT=ctxT[:, k, ss],
                            rhs=wo_sb[:, k, 0:512],
                            start=(k == 0), stop=(k == NPAIR - 1),
                        )
                        nc.tensor.matmul(
                            ops[:, 512:E], lhsT=ctxT[:, k, ss],
                            rhs=wo_sb[:, k, 512:E],
                            start=(k == 0), stop=(k == NPAIR - 1),
                        )
                    for oc in range(2):
                        ysb = outp.tile([128, 512], f32, tag="ysb")
                        nc.vector.tensor_copy(ysb[:], ops[:, oc * 512:(oc + 1) * 512])
                        nc.sync.dma_start(
                            out=y[ss, oc * 512:(oc + 1) * 512], in_=ysb[:]
                        )

    nc.compile()
    return nc


def _get_program():
    if "nc" not in _CACHE:
        _CACHE["nc"] = _build_program()
    return _CACHE["nc"]


def _host_prep(query, Wq, Wk, Wv, Wo):
    bf16 = ml_dtypes.bfloat16
    q_bf = np.ascontiguousarray(query.astype(bf16))
    wq_bd = np.zeros((NPAIR, 128, 128), dtype=bf16)
    wkv_bd = np.zeros((NPAIR, 128, 256), dtype=bf16)
    for p in range(NPAIR):
        wq_bd[p, 0:64, 0:64] = Wq[2 * p]
        wq_bd[p, 64:128, 64:128] = Wq[2 * p + 1]
        wkv_bd[p, 0:64, 0:64] = Wk[2 * p]
        wkv_bd[p, 64:128, 64:128] = Wk[2 * p + 1]
        wkv_bd[p, 0:64, 128:192] = Wv[2 * p]
        wkv_bd[p, 64:128, 192:256] = Wv[2 * p + 1]
    wo_bf = np.ascontiguousarray(Wo.astype(bf16))
    in_maps = []
    for c in range(N_CORES):
        b, j = divmod(c, 2)
        in_maps.append({
            "xq": np.ascontiguousarray(q_bf[b, j * SL:(j + 1) * SL, :]),
            "wq_bd": wq_bd,
            "wkv_bd": wkv_bd,
            "wo": wo_bf,
        })
    return in_maps


def kernel(query, Wq, Wk, Wv, Wo):
    from concourse.bass_utils import run_bass_kernel_spmd

    nc = _get_program()
    in_maps = _host_prep(query, Wq, Wk, Wv, Wo)
    res = run_bass_kernel_spmd(nc, in_maps, list(range(N_CORES)))
    out = np.empty((B, S, E), dtype=np.float32)
    for c in range(N_CORES):
        b, j = divmod(c, 2)
        out[b, j * SL:(j + 1) * SL, :] = res.results[c]["y"]
    return out


# revision 27
# speedup vs baseline: 1.0645x; 1.0080x over previous
"""Multi-head linear attention (elu+1 feature map) on 8 TRN2 NeuronCores.

Sharding: core c handles batch b = c//2, sequence half j = c%2 (2048 rows).
Each core computes q/k/v projections + phi + partial kv/z for its rows,
AllReduces kv/z across the (b, j) pair, then computes num/den/ctx and the
output projection for its rows. All matmuls in bf16 (fp32 PSUM accumulate).

Device-side layout notes:
  - query slice is DMA-transposed (bf16 xbar) into feature-major xT
    pair-blocks (128 = 2 heads x 64 dims, S) so projections contract the
    feature dim on partitions with block-diagonal weights.
  - kf/v are produced sequence-major (s on partitions) by using xT chunks
    as the matmul stationary; kv/z accumulate in PSUM over s-chunks.
  - phi(x) = elu(x)+1 is computed exactly as max(x+1, exp(min(x+1,1)-1)).
  - num/den use feature-major qfT; den rows for all 16 heads accumulate in
    one PSUM tile via zero-padded z columns; division by den is fused into
    the num PSUM eviction against a partition-broadcast reciprocal.
  - output projection consumes feature-major ctxT as stationary and natural
    Wo row-blocks as moving operand, yielding sequence-major output.
"""
import numpy as np
import ml_dtypes

B, S, H, Dh = 4, 4096, 16, 64
E = H * Dh
N_CORES = 8
SL = S // 2          # sequence rows per core
NPAIR = H // 2       # head pairs
EPS = 1e-6

_CACHE = {}


def _build_program():
    import concourse.bacc as bacc
    import concourse.mybir as mybir
    import concourse.tile as tile

    bf16 = mybir.dt.bfloat16
    f32 = mybir.dt.float32
    Act = mybir.ActivationFunctionType
    Alu = mybir.AluOpType

    nc = bacc.Bacc(None, target_bir_lowering=False, num_devices=N_CORES)

    xq = nc.dram_tensor("xq", [SL, E], bf16, kind="ExternalInput")
    wq_bd = nc.dram_tensor("wq_bd", [NPAIR, 128, 128], bf16, kind="ExternalInput")
    wkv_bd = nc.dram_tensor("wkv_bd", [NPAIR, 128, 256], bf16, kind="ExternalInput")
    wo = nc.dram_tensor("wo", [E, E], bf16, kind="ExternalInput")
    y = nc.dram_tensor("y", [SL, E], f32, kind="ExternalOutput")
    kv_ar = nc.dram_tensor("kv_ar", [128, NPAIR * 129], f32)

    NCHUNK = SL // 128   # s-chunks per pair for kf/v (16)
    NQC = SL // 512      # 512-wide chunks for qf / num / den (4)
    NSO = SL // 128      # output row chunks (16)

    with tile.TileContext(nc) as tc:
        with (
            tc.tile_pool(name="persist", bufs=1) as persist,
            tc.tile_pool(name="xp", bufs=2) as xp,
            tc.tile_pool(name="kvsb", bufs=2) as kvsb,
            tc.tile_pool(name="tmp", bufs=2) as tmp,
            tc.tile_pool(name="rbcp", bufs=2) as rbcp,
            tc.tile_pool(name="outp", bufs=2) as outp,
            tc.tile_pool(name="dram", bufs=1, space="DRAM") as dram,
        ):
            # ---- weights / constants ----
            # Load order matters for startup latency: the first K1 matmul
            # needs only wkv + xT[0], so those go first on the sync queue;
            # the big Wo block rides the SWDGE (gpsimd) queues in parallel.
            wkv_sb = persist.tile([128, NPAIR, 256], bf16)
            nc.sync.dma_start(out=wkv_sb[:], in_=wkv_bd.rearrange("p k m -> k p m"))
            # All 8 xT pair blocks stay resident (32 KiB/partition total).
            xTs = []
            for p in range(NPAIR):
                xT = xp.tile([128, SL], bf16, tag=f"xT{p}")
                nc.sync.dma_start(
                    out=xT[:], in_=xq[:, p * 128:(p + 1) * 128], transpose=True
                )
                xTs.append(xT)
            wq_sb = persist.tile([128, NPAIR, 128], bf16)
            nc.gpsimd.dma_start(out=wq_sb[:], in_=wq_bd.rearrange("p k m -> k p m"))
            wo_sb = persist.tile([128, NPAIR, E], bf16)
            nc.gpsimd.dma_start(
                out=wo_sb[:], in_=wo.rearrange("(k p) n -> p k n", p=128)
            )
            qfT = persist.tile([128, NPAIR, SL], bf16)
            ctxT = persist.tile([128, NPAIR, SL], bf16)

            # ---- phase K1: kf/v (s-major) + kv/z for every pair ----
            kv_in = dram.tile([128, NPAIR * 129], f32)
            z_in = dram.tile([128, NPAIR], f32)
            with (
                tc.tile_pool(name="ps_kvp", bufs=2, space="PSUM") as ps_kvp,
                tc.tile_pool(name="ps_kv", bufs=2, space="PSUM") as ps_kv,
            ):
                for p in range(NPAIR):
                    xT = xTs[p]
                    kf = kvsb.tile([128, NCHUNK, 128], bf16, tag="kf")
                    vsb = kvsb.tile([128, NCHUNK, 130], bf16, tag="v")
                    nc.vector.memset(vsb[:, :, 128:129], 1.0)
                    kvacc = ps_kv.tile([128, 130], f32, tag="kvacc")
                    for g in range(NCHUNK // 4):
                        kvps = ps_kvp.tile([128, 1024], f32, tag="kvps")
                        kvps3 = kvps[:].rearrange("q (c x) -> q c x", c=4)
                        for c4 in range(4):
                            i = g * 4 + c4
                            nc.tensor.matmul(
                                kvps3[:, c4, :],
                                lhsT=xT[:, i * 128:(i + 1) * 128],
                                rhs=wkv_sb[:, p, :],
                                start=True, stop=True,
                            )
                        cs = slice(g * 4, (g + 1) * 4)
                        kA = tmp.tile([128, 4, 128], bf16, tag="kA")
                        nc.vector.tensor_scalar_add(kA[:], kvps3[:, :, 0:128], 1.0)
                        nc.vector.tensor_copy(vsb[:, cs, 0:128], kvps3[:, :, 128:256])
                        kM = tmp.tile([128, 4, 128], bf16, tag="kM")
                        nc.vector.tensor_scalar(
                            kM[:], kA[:], 1.0, -1.0, Alu.min, Alu.add
                        )
                        nc.scalar.activation(kM[:], kM[:], Act.Exp)
                        nc.vector.tensor_tensor(kf[:, cs, :], kA[:], kM[:], Alu.max)
                        for c4 in range(4):
                            i = g * 4 + c4
                            nc.tensor.matmul(
                                kvacc[:, 0:129],
                                lhsT=kf[:, i, :], rhs=vsb[:, i, 0:129],
                                start=(i == 0), stop=(i == NCHUNK - 1),
                            )
                    kvst = outp.tile([128, 129], f32, tag="kvst")
                    nc.vector.tensor_copy(kvst[:], kvacc[:, 0:129])
                    nc.sync.dma_start(
                        out=kv_in[:, p * 129:(p + 1) * 129], in_=kvst[:]
                    )
                    nc.sync.dma_start(
                        out=z_in[:, p:p + 1], in_=kvst[:, 128:129]
                    )

            # ---- phase R: z-only AllReduce first (small, unblocks den),
            # then the kv AllReduce; both overlap phase K2 below ----
            groups = [[0, 1], [2, 3], [4, 5], [6, 7]]
            z_ar = nc.dram_tensor("z_ar", [128, NPAIR], f32)
            nc.gpsimd.collective_compute(
                "AllReduce", Alu.add, replica_groups=groups,
                ins=[z_in[:]], outs=[z_ar[:]],
            )
            nc.gpsimd.collective_compute(
                "AllReduce", Alu.add, replica_groups=groups,
                ins=[kv_in[:]], outs=[kv_ar[:]],
            )
            z_rd = persist.tile([128, NPAIR], f32)
            nc.sync.dma_start(out=z_rd[:], in_=z_ar[:])
            # Per-pair zero-padded z columns for the den accumulation.
            zbd = persist.tile([128, NPAIR, H], bf16)
            nc.vector.memset(zbd[:], 0.0)
            for p in range(NPAIR):
                nc.vector.tensor_copy(
                    zbd[0:64, p, 2 * p:2 * p + 1], z_rd[0:64, p:p + 1]
                )
                nc.vector.tensor_copy(
                    zbd[64:128, p, 2 * p + 1:2 * p + 2], z_rd[64:128, p:p + 1]
                )

            # ---- phase K2: qf (feature-major) + den accumulation, both
            # overlapping the kv collective ----
            with (
                tc.tile_pool(name="ps_q", bufs=2, space="PSUM") as ps_q,
                tc.tile_pool(name="ps_den", bufs=1, space="PSUM") as ps_den,
            ):
                denps = ps_den.tile([16, SL], f32)
                for p in range(NPAIR):
                    xT = xTs[p]
                    for qc in range(2):
                        qs = slice(qc * 1024, (qc + 1) * 1024)
                        qps = ps_q.tile([128, 1024], f32, tag="qps")
                        nc.tensor.matmul(
                            qps[:, 0:512], lhsT=wq_sb[:, p, :],
                            rhs=xT[:, qc * 1024:qc * 1024 + 512],
                            start=True, stop=True,
                        )
                        nc.tensor.matmul(
                            qps[:, 512:1024], lhsT=wq_sb[:, p, :],
                            rhs=xT[:, qc * 1024 + 512:(qc + 1) * 1024],
                            start=True, stop=True,
                        )
                        qA = tmp.tile([128, 1024], bf16, tag="qA")
                        nc.scalar.activation(qA[:], qps[:], Act.Identity, bias=1.0)
                        qM = tmp.tile([128, 1024], bf16, tag="qM")
                        nc.vector.tensor_scalar(
                            qM[:], qA[:], 1.0, -1.0, Alu.min, Alu.add
                        )
                        nc.scalar.activation(qM[:], qM[:], Act.Exp)
                        nc.vector.tensor_tensor(qfT[:, p, qs], qA[:], qM[:], Alu.max)
                    for qc in range(NQC):
                        qs = slice(qc * 512, (qc + 1) * 512)
                        nc.tensor.matmul(
                            denps[:, qs], lhsT=zbd[:, p, :], rhs=qfT[:, p, qs],
                            start=(p == 0), stop=(p == NPAIR - 1),
                        )
                den_sb = persist.tile([16, SL], f32)
                nc.vector.tensor_scalar_add(den_sb[:], denps[:], EPS)
            kvrd = persist.tile([128, NPAIR, 129], f32)
            nc.sync.dma_start(
                out=kvrd[:], in_=kv_ar.rearrange("q (p c) -> q p c", c=129)
            )

            # ---- reciprocal of den, computed in a (128, 256) layout so all
            # DVE lanes participate (a (16, SL) reciprocal is 8x slower) ----
            den_dram = dram.tile([16, SL], f32)
            nc.sync.dma_start(out=den_dram[:], in_=den_sb[:])
            den128 = persist.tile([128, SL // 8], f32)
            nc.sync.dma_start(
                out=den128[:],
                in_=den_dram[:].rearrange("h (g c) -> (h g) c", c=SL // 8),
            )
            nc.vector.reciprocal(den128[:], den128[:])
            recip_bf = persist.tile([128, SL // 8], bf16)
            nc.vector.tensor_copy(recip_bf[:], den128[:])
            recip_dram = dram.tile([16, SL], bf16)
            nc.sync.dma_start(
                out=recip_dram[:].rearrange("h (g c) -> (h g) c", c=SL // 8),
                in_=recip_bf[:],
            )

            # ---- phase N2: num + fused divide -> ctxT ----
            with tc.tile_pool(name="ps_num", bufs=3, space="PSUM") as ps_num:
                for p in range(NPAIR):
                    kvbd = tmp.tile([128, 128], bf16, tag="kvbd")
                    nc.vector.memset(kvbd[:], 0.0)
                    nc.vector.tensor_copy(kvbd[0:64, 0:64], kvrd[0:64, p, 0:64])
                    nc.vector.tensor_copy(
                        kvbd[64:128, 64:128], kvrd[64:128, p, 64:128]
                    )
                    for qc in range(2):
                        qs = slice(qc * 1024, (qc + 1) * 1024)
                        rbc = rbcp.tile([128, 1024], bf16, tag="rbc")
                        nc.sync.dma_start(
                            out=rbc[0:64, :],
                            in_=recip_dram[2 * p:2 * p + 1, qs].to_broadcast(
                                [64, 1024]
                            ),
                        )
                        nc.sync.dma_start(
                            out=rbc[64:128, :],
                            in_=recip_dram[2 * p + 1:2 * p + 2, qs].to_broadcast(
                                [64, 1024]
                            ),
                        )
                        nps = ps_num.tile([128, 1024], f32, tag="nps")
                        nc.tensor.matmul(
                            nps[:, 0:512], lhsT=kvbd[:],
                            rhs=qfT[:, p, qc * 1024:qc * 1024 + 512],
                            start=True, stop=True,
                        )
                        nc.tensor.matmul(
                            nps[:, 512:1024], lhsT=kvbd[:],
                            rhs=qfT[:, p, qc * 1024 + 512:(qc + 1) * 1024],
                            start=True, stop=True,
                        )
                        nc.vector.tensor_tensor(
                            ctxT[:, p, qs], nps[:], rbc[:], Alu.mult
                        )

            # ---- phase O: output projection (sequence-major out) ----
            with tc.tile_pool(name="ps_o", bufs=2, space="PSUM") as ps_o:
                for si in range(NSO):
                    ss = slice(si * 128, (si + 1) * 128)
                    ops = ps_o.tile([128, E], f32, tag="ops")
                    for k in range(NPAIR):
                        nc.tensor.matmul(
                            ops[:, 0:512], lhsT=ctxT[:, k, ss],
                            rhs=wo_sb[:, k, 0:512],
                            start=(k == 0), stop=(k == NPAIR - 1),
                        )
                        nc.tensor.matmul(
                            ops[:, 512:E], lhsT=ctxT[:, k, ss],
                            rhs=wo_sb[:, k, 512:E],
                            start=(k == 0), stop=(k == NPAIR - 1),
                        )
                    for oc in range(2):
                        ysb = outp.tile([128, 512], f32, tag="ysb")
                        nc.scalar.copy(ysb[:], ops[:, oc * 512:(oc + 1) * 512])
                        nc.sync.dma_start(
                            out=y[ss, oc * 512:(oc + 1) * 512], in_=ysb[:]
                        )

    nc.compile()
    return nc


def _get_program():
    if "nc" not in _CACHE:
        _CACHE["nc"] = _build_program()
    return _CACHE["nc"]


def _host_prep(query, Wq, Wk, Wv, Wo):
    bf16 = ml_dtypes.bfloat16
    q_bf = np.ascontiguousarray(query.astype(bf16))
    wq_bd = np.zeros((NPAIR, 128, 128), dtype=bf16)
    wkv_bd = np.zeros((NPAIR, 128, 256), dtype=bf16)
    for p in range(NPAIR):
        wq_bd[p, 0:64, 0:64] = Wq[2 * p]
        wq_bd[p, 64:128, 64:128] = Wq[2 * p + 1]
        wkv_bd[p, 0:64, 0:64] = Wk[2 * p]
        wkv_bd[p, 64:128, 64:128] = Wk[2 * p + 1]
        wkv_bd[p, 0:64, 128:192] = Wv[2 * p]
        wkv_bd[p, 64:128, 192:256] = Wv[2 * p + 1]
    wo_bf = np.ascontiguousarray(Wo.astype(bf16))
    in_maps = []
    for c in range(N_CORES):
        b, j = divmod(c, 2)
        in_maps.append({
            "xq": np.ascontiguousarray(q_bf[b, j * SL:(j + 1) * SL, :]),
            "wq_bd": wq_bd,
            "wkv_bd": wkv_bd,
            "wo": wo_bf,
        })
    return in_maps


def kernel(query, Wq, Wk, Wv, Wo):
    from concourse.bass_utils import run_bass_kernel_spmd

    nc = _get_program()
    in_maps = _host_prep(query, Wq, Wk, Wv, Wo)
    res = run_bass_kernel_spmd(nc, in_maps, list(range(N_CORES)))
    out = np.empty((B, S, E), dtype=np.float32)
    for c in range(N_CORES):
        b, j = divmod(c, 2)
        out[b, j * SL:(j + 1) * SL, :] = res.results[c]["y"]
    return out


# revision 29
# speedup vs baseline: 1.1067x; 1.0397x over previous
"""Multi-head linear attention (elu+1 feature map) on 8 TRN2 NeuronCores.

Sharding: core c handles batch b = c//2, sequence half j = c%2 (2048 rows).
Each core computes q/k/v projections + phi + partial kv/z for its rows,
AllReduces kv/z across the (b, j) pair, then computes num/den/ctx and the
output projection for its rows. All matmuls in bf16 (fp32 PSUM accumulate).

Device-side layout notes:
  - query slice is DMA-transposed (bf16 xbar) into feature-major xT
    pair-blocks (128 = 2 heads x 64 dims, S) so projections contract the
    feature dim on partitions with block-diagonal weights.
  - kf/v are produced sequence-major (s on partitions) by using xT chunks
    as the matmul stationary; kv/z accumulate in PSUM over s-chunks.
  - phi(x) = elu(x)+1 is computed exactly as max(x+1, exp(min(x+1,1)-1)).
  - num/den use feature-major qfT; den rows for all 16 heads accumulate in
    one PSUM tile via zero-padded z columns; division by den is fused into
    the num PSUM eviction against a partition-broadcast reciprocal.
  - output projection consumes feature-major ctxT as stationary and natural
    Wo row-blocks as moving operand, yielding sequence-major output.
"""
import numpy as np
import ml_dtypes

B, S, H, Dh = 4, 4096, 16, 64
E = H * Dh
N_CORES = 8
SL = S // 2          # sequence rows per core
NPAIR = H // 2       # head pairs
EPS = 1e-6

_CACHE = {}


def _build_program():
    import concourse.bacc as bacc
    import concourse.mybir as mybir
    import concourse.tile as tile

    bf16 = mybir.dt.bfloat16
    f32 = mybir.dt.float32
    Act = mybir.ActivationFunctionType
    Alu = mybir.AluOpType

    nc = bacc.Bacc(None, target_bir_lowering=False, num_devices=N_CORES)

    xq = nc.dram_tensor("xq", [SL, E], bf16, kind="ExternalInput")
    wq_bd = nc.dram_tensor("wq_bd", [NPAIR, 128, 128], bf16, kind="ExternalInput")
    wkv_bd = nc.dram_tensor("wkv_bd", [NPAIR, 128, 256], bf16, kind="ExternalInput")
    wo = nc.dram_tensor("wo", [E, E], bf16, kind="ExternalInput")
    y = nc.dram_tensor("y", [SL, E], f32, kind="ExternalOutput")
    kv_ar = nc.dram_tensor("kv_ar", [128, NPAIR * 129], f32)

    NCHUNK = SL // 128   # s-chunks per pair for kf/v (16)
    NQC = SL // 512      # 512-wide chunks for qf / num / den (4)
    NSO = SL // 128      # output row chunks (16)

    with tile.TileContext(nc) as tc:
        with (
            tc.tile_pool(name="persist", bufs=1) as persist,
            tc.tile_pool(name="xp", bufs=2) as xp,
            tc.tile_pool(name="kvsb", bufs=2) as kvsb,
            tc.tile_pool(name="tmp", bufs=3) as tmp,
            tc.tile_pool(name="rbcp", bufs=2) as rbcp,
            tc.tile_pool(name="outp", bufs=2) as outp,
            tc.tile_pool(name="dram", bufs=1, space="DRAM") as dram,
        ):
            # ---- weights / constants ----
            # Load order matters for startup latency: the first K1 matmul
            # needs only wkv + xT[0], so those go first on the sync queue;
            # the big Wo block rides the SWDGE (gpsimd) queues in parallel.
            wkv_sb = persist.tile([128, NPAIR, 256], bf16)
            nc.sync.dma_start(out=wkv_sb[:], in_=wkv_bd.rearrange("p k m -> k p m"))
            # All 8 xT pair blocks stay resident (32 KiB/partition total).
            # Transposes go before any other bulk DMA: the xbar-mode switch
            # serializes DMA_TRANSPOSE behind in-flight copies.
            xTs = []
            for p in range(NPAIR):
                xT = xp.tile([128, SL], bf16, tag=f"xT{p}")
                nc.sync.dma_start(
                    out=xT[:], in_=xq[:, p * 128:(p + 1) * 128], transpose=True
                )
                xTs.append(xT)
            wq_sb = persist.tile([128, NPAIR, 128], bf16)
            nc.gpsimd.dma_start(out=wq_sb[:], in_=wq_bd.rearrange("p k m -> k p m"))
            wo_sb = persist.tile([128, NPAIR, E], bf16)
            nc.gpsimd.dma_start(
                out=wo_sb[:], in_=wo.rearrange("(k p) n -> p k n", p=128)
            )
            qfT = persist.tile([128, NPAIR, SL], bf16)
            ctxT = persist.tile([128, NPAIR, SL], bf16)

            # ---- phase K1: kf/v (s-major) + kv/z for every pair ----
            kv_in = dram.tile([128, NPAIR * 129], f32)
            z_in = dram.tile([128, NPAIR], f32)
            with (
                tc.tile_pool(name="ps_kvp", bufs=2, space="PSUM") as ps_kvp,
                tc.tile_pool(name="ps_kv", bufs=2, space="PSUM") as ps_kv,
            ):
                for p in range(NPAIR):
                    xT = xTs[p]
                    kf = kvsb.tile([128, NCHUNK, 128], bf16, tag="kf")
                    vsb = kvsb.tile([128, NCHUNK, 130], bf16, tag="v")
                    nc.vector.memset(vsb[:, :, 128:129], 1.0)
                    kvacc = ps_kv.tile([128, 130], f32, tag="kvacc")
                    for g in range(NCHUNK // 4):
                        kvps = ps_kvp.tile([128, 1024], f32, tag="kvps")
                        kvps3 = kvps[:].rearrange("q (c x) -> q c x", c=4)
                        for c4 in range(4):
                            i = g * 4 + c4
                            nc.tensor.matmul(
                                kvps3[:, c4, :],
                                lhsT=xT[:, i * 128:(i + 1) * 128],
                                rhs=wkv_sb[:, p, :],
                                start=True, stop=True,
                            )
                        cs = slice(g * 4, (g + 1) * 4)
                        kA = tmp.tile([128, 4, 128], bf16, tag="kA")
                        nc.scalar.activation(
                            kA[:], kvps3[:, :, 0:128], Act.Identity, bias=1.0
                        )
                        nc.vector.tensor_copy(vsb[:, cs, 0:128], kvps3[:, :, 128:256])
                        kM = tmp.tile([128, 4, 128], bf16, tag="kM")
                        nc.vector.tensor_scalar(
                            kM[:], kA[:], 1.0, -1.0, Alu.min, Alu.add
                        )
                        nc.scalar.activation(kM[:], kM[:], Act.Exp)
                        nc.vector.tensor_tensor(kf[:, cs, :], kA[:], kM[:], Alu.max)
                        for c4 in range(4):
                            i = g * 4 + c4
                            nc.tensor.matmul(
                                kvacc[:, 0:129],
                                lhsT=kf[:, i, :], rhs=vsb[:, i, 0:129],
                                start=(i == 0), stop=(i == NCHUNK - 1),
                            )
                    kvst = outp.tile([128, 129], f32, tag="kvst")
                    nc.vector.tensor_copy(kvst[:], kvacc[:, 0:129])
                    nc.sync.dma_start(
                        out=kv_in[:, p * 129:(p + 1) * 129], in_=kvst[:]
                    )
                    nc.sync.dma_start(
                        out=z_in[:, p:p + 1], in_=kvst[:, 128:129]
                    )

            # ---- phase R: z-only AllReduce first (small, unblocks den),
            # then the kv AllReduce; both overlap phase K2 below ----
            groups = [[0, 1], [2, 3], [4, 5], [6, 7]]
            z_ar = nc.dram_tensor("z_ar", [128, NPAIR], f32)
            nc.gpsimd.collective_compute(
                "AllReduce", Alu.add, replica_groups=groups,
                ins=[z_in[:]], outs=[z_ar[:]],
            )
            nc.gpsimd.collective_compute(
                "AllReduce", Alu.add, replica_groups=groups,
                ins=[kv_in[:]], outs=[kv_ar[:]],
            )
            z_rd = persist.tile([128, NPAIR], f32)
            nc.sync.dma_start(out=z_rd[:], in_=z_ar[:])
            # Per-pair zero-padded z columns for the den accumulation.
            zbd = persist.tile([128, NPAIR, H], bf16)
            nc.vector.memset(zbd[:], 0.0)
            for p in range(NPAIR):
                nc.vector.tensor_copy(
                    zbd[0:64, p, 2 * p:2 * p + 1], z_rd[0:64, p:p + 1]
                )
                nc.vector.tensor_copy(
                    zbd[64:128, p, 2 * p + 1:2 * p + 2], z_rd[64:128, p:p + 1]
                )

            # ---- phase K2: qf (feature-major) + den accumulation, both
            # overlapping the kv collective ----
            with (
                tc.tile_pool(name="ps_q", bufs=2, space="PSUM") as ps_q,
                tc.tile_pool(name="ps_den", bufs=1, space="PSUM") as ps_den,
            ):
                denps = ps_den.tile([16, SL], f32)
                for p in range(NPAIR):
                    xT = xTs[p]
                    for qc in range(2):
                        qs = slice(qc * 1024, (qc + 1) * 1024)
                        qps = ps_q.tile([128, 1024], f32, tag="qps")
                        nc.tensor.matmul(
                            qps[:, 0:512], lhsT=wq_sb[:, p, :],
                            rhs=xT[:, qc * 1024:qc * 1024 + 512],
                            start=True, stop=True,
                        )
                        nc.tensor.matmul(
                            qps[:, 512:1024], lhsT=wq_sb[:, p, :],
                            rhs=xT[:, qc * 1024 + 512:(qc + 1) * 1024],
                            start=True, stop=True,
                        )
                        qA = tmp.tile([128, 1024], bf16, tag="qA")
                        nc.scalar.activation(qA[:], qps[:], Act.Identity, bias=1.0)
                        qM = tmp.tile([128, 1024], bf16, tag="qM")
                        nc.vector.tensor_scalar(
                            qM[:], qA[:], 1.0, -1.0, Alu.min, Alu.add
                        )
                        nc.scalar.activation(qM[:], qM[:], Act.Exp)
                        nc.vector.tensor_tensor(qfT[:, p, qs], qA[:], qM[:], Alu.max)
                    for qc in range(NQC):
                        qs = slice(qc * 512, (qc + 1) * 512)
                        nc.tensor.matmul(
                            denps[:, qs], lhsT=zbd[:, p, :], rhs=qfT[:, p, qs],
                            start=(p == 0), stop=(p == NPAIR - 1),
                        )
                den_sb = persist.tile([16, SL], bf16)
                eps_sb = persist.tile([16, 1], f32)
                nc.vector.memset(eps_sb[:], EPS)
                nc.scalar.activation(
                    den_sb[:], denps[:], Act.Identity, bias=eps_sb[:]
                )
            kvrd = persist.tile([128, NPAIR, 129], f32)
            nc.sync.dma_start(
                out=kvrd[:], in_=kv_ar.rearrange("q (p c) -> q p c", c=129)
            )

            # ---- reciprocal of den on the Scalar engine LUT (one pass; its
            # ~1e-3 accuracy is invisible next to the bf16 quantization of the
            # reciprocal that follows) ----
            recip_bf = persist.tile([16, SL], bf16)
            eng = nc.scalar
            eng.add_instruction(
                mybir.InstActivation(
                    name=nc.get_next_instruction_name(),
                    func=Act.Reciprocal,
                    ins=[
                        eng.lower_ap(den_sb[:]),
                        mybir.ImmediateValue(dtype=f32, value=0.0),
                        mybir.ImmediateValue(dtype=f32, value=1.0),
                        mybir.ImmediateValue(dtype=f32, value=0.0),
                    ],
                    outs=[eng.lower_ap(recip_bf[:])],
                )
            )
            recip_dram = dram.tile([16, SL], bf16)
            nc.sync.dma_start(out=recip_dram[:], in_=recip_bf[:])

            # ---- phase N2: num + fused divide -> ctxT ----
            with tc.tile_pool(name="ps_num", bufs=3, space="PSUM") as ps_num:
                for p in range(NPAIR):
                    kvbd = tmp.tile([128, 128], bf16, tag="kvbd")
                    nc.vector.memset(kvbd[:], 0.0)
                    nc.vector.tensor_copy(kvbd[0:64, 0:64], kvrd[0:64, p, 0:64])
                    nc.vector.tensor_copy(
                        kvbd[64:128, 64:128], kvrd[64:128, p, 64:128]
                    )
                    for qc in range(2):
                        qs = slice(qc * 1024, (qc + 1) * 1024)
                        rbc = rbcp.tile([128, 1024], bf16, tag="rbc")
                        nc.sync.dma_start(
                            out=rbc[0:64, :],
                            in_=recip_dram[2 * p:2 * p + 1, qs].to_broadcast(
                                [64, 1024]
                            ),
                        )
                        nc.sync.dma_start(
                            out=rbc[64:128, :],
                            in_=recip_dram[2 * p + 1:2 * p + 2, qs].to_broadcast(
                                [64, 1024]
                            ),
                        )
                        nps = ps_num.tile([128, 1024], f32, tag="nps")
                        nc.tensor.matmul(
                            nps[:, 0:512], lhsT=kvbd[:],
                            rhs=qfT[:, p, qc * 1024:qc * 1024 + 512],
                            start=True, stop=True,
                        )
                        nc.tensor.matmul(
                            nps[:, 512:1024], lhsT=kvbd[:],
                            rhs=qfT[:, p, qc * 1024 + 512:(qc + 1) * 1024],
                            start=True, stop=True,
                        )
                        nc.vector.tensor_tensor(
                            ctxT[:, p, qs], nps[:], rbc[:], Alu.mult
                        )

            # ---- phase O: output projection (sequence-major out) ----
            with tc.tile_pool(name="ps_o", bufs=2, space="PSUM") as ps_o:
                for si in range(NSO):
                    ss = slice(si * 128, (si + 1) * 128)
                    ops = ps_o.tile([128, E], f32, tag="ops")
                    for k in range(NPAIR):
                        nc.tensor.matmul(
                            ops[:, 0:512], lhsT=ctxT[:, k, ss],
                            rhs=wo_sb[:, k, 0:512],
                            start=(k == 0), stop=(k == NPAIR - 1),
                        )
                        nc.tensor.matmul(
                            ops[:, 512:E], lhsT=ctxT[:, k, ss],
                            rhs=wo_sb[:, k, 512:E],
                            start=(k == 0), stop=(k == NPAIR - 1),
                        )
                    for oc in range(2):
                        ysb = outp.tile([128, 512], f32, tag="ysb")
                        nc.scalar.copy(ysb[:], ops[:, oc * 512:(oc + 1) * 512])
                        nc.sync.dma_start(
                            out=y[ss, oc * 512:(oc + 1) * 512], in_=ysb[:]
                        )

    nc.compile()
    return nc


def _get_program():
    if "nc" not in _CACHE:
        _CACHE["nc"] = _build_program()
    return _CACHE["nc"]


def _host_prep(query, Wq, Wk, Wv, Wo):
    bf16 = ml_dtypes.bfloat16
    q_bf = np.ascontiguousarray(query.astype(bf16))
    wq_bd = np.zeros((NPAIR, 128, 128), dtype=bf16)
    wkv_bd = np.zeros((NPAIR, 128, 256), dtype=bf16)
    for p in range(NPAIR):
        wq_bd[p, 0:64, 0:64] = Wq[2 * p]
        wq_bd[p, 64:128, 64:128] = Wq[2 * p + 1]
        wkv_bd[p, 0:64, 0:64] = Wk[2 * p]
        wkv_bd[p, 64:128, 64:128] = Wk[2 * p + 1]
        wkv_bd[p, 0:64, 128:192] = Wv[2 * p]
        wkv_bd[p, 64:128, 192:256] = Wv[2 * p + 1]
    wo_bf = np.ascontiguousarray(Wo.astype(bf16))
    in_maps = []
    for c in range(N_CORES):
        b, j = divmod(c, 2)
        in_maps.append({
            "xq": np.ascontiguousarray(q_bf[b, j * SL:(j + 1) * SL, :]),
            "wq_bd": wq_bd,
            "wkv_bd": wkv_bd,
            "wo": wo_bf,
        })
    return in_maps


def kernel(query, Wq, Wk, Wv, Wo):
    from concourse.bass_utils import run_bass_kernel_spmd

    nc = _get_program()
    in_maps = _host_prep(query, Wq, Wk, Wv, Wo)
    res = run_bass_kernel_spmd(nc, in_maps, list(range(N_CORES)))
    out = np.empty((B, S, E), dtype=np.float32)
    for c in range(N_CORES):
        b, j = divmod(c, 2)
        out[b, j * SL:(j + 1) * SL, :] = res.results[c]["y"]
    return out


# revision 31
# speedup vs baseline: 1.2075x; 1.0910x over previous
"""Multi-head linear attention (elu+1 feature map) on 8 TRN2 NeuronCores.

Sharding: core c handles batch b = c//2, sequence half j = c%2 (2048 rows).
Each core computes q/k/v projections + phi + partial kv/z for its rows,
AllReduces kv/z across the (b, j) pair, then computes num/den/ctx and the
output projection for its rows. All matmuls in bf16 (fp32 PSUM accumulate).

Device-side layout notes:
  - query slice is DMA-transposed (bf16 xbar) into feature-major xT
    pair-blocks (128 = 2 heads x 64 dims, S) so projections contract the
    feature dim on partitions with block-diagonal weights.
  - kf/v are produced sequence-major (s on partitions) by using xT chunks
    as the matmul stationary; kv/z accumulate in PSUM over s-chunks.
  - phi(x) = elu(x)+1 is computed exactly as max(x+1, exp(min(x+1,1)-1)).
  - num/den use feature-major qfT; den rows for all 16 heads accumulate in
    one PSUM tile via zero-padded z columns; division by den is fused into
    the num PSUM eviction against a partition-broadcast reciprocal.
  - output projection consumes feature-major ctxT as stationary and natural
    Wo row-blocks as moving operand, yielding sequence-major output.
"""
import numpy as np
import ml_dtypes

B, S, H, Dh = 4, 4096, 16, 64
E = H * Dh
N_CORES = 8
SL = S // 2          # sequence rows per core
NPAIR = H // 2       # head pairs
EPS = 1e-6

_CACHE = {}


def _build_program():
    import concourse.bacc as bacc
    import concourse.mybir as mybir
    import concourse.tile as tile

    bf16 = mybir.dt.bfloat16
    f32 = mybir.dt.float32
    Act = mybir.ActivationFunctionType
    Alu = mybir.AluOpType

    nc = bacc.Bacc(None, target_bir_lowering=False, num_devices=N_CORES)

    xq = nc.dram_tensor("xqT", [E, SL], bf16, kind="ExternalInput")
    wq_bd = nc.dram_tensor("wq_bd", [NPAIR, 128, 128], bf16, kind="ExternalInput")
    wkv_bd = nc.dram_tensor("wkv_bd", [NPAIR, 128, 256], bf16, kind="ExternalInput")
    wo = nc.dram_tensor("wo", [E, E], bf16, kind="ExternalInput")
    y = nc.dram_tensor("y", [SL, E], f32, kind="ExternalOutput")
    kv_ar = nc.dram_tensor("kv_ar", [128, NPAIR * 129], f32)

    NCHUNK = SL // 128   # s-chunks per pair for kf/v (16)
    NQC = SL // 512      # 512-wide chunks for qf / num / den (4)
    NSO = SL // 128      # output row chunks (16)

    with tile.TileContext(nc) as tc:
        with (
            tc.tile_pool(name="persist", bufs=1) as persist,
            tc.tile_pool(name="xp", bufs=2) as xp,
            tc.tile_pool(name="kvsb", bufs=2) as kvsb,
            tc.tile_pool(name="tmp", bufs=3) as tmp,
            tc.tile_pool(name="rbcp", bufs=2) as rbcp,
            tc.tile_pool(name="outp", bufs=2) as outp,
            tc.tile_pool(name="dram", bufs=1, space="DRAM") as dram,
        ):
            # ---- weights / constants ----
            # Load order matters for startup latency: the first K1 matmul
            # needs only wkv + xT[0], so those go first on the sync queue;
            # the big Wo block rides the SWDGE (gpsimd) queues in parallel.
            wkv_sb = persist.tile([128, NPAIR, 256], bf16)
            nc.sync.dma_start(out=wkv_sb[:], in_=wkv_bd.rearrange("p k m -> k p m"))
            # All 8 xT pair blocks stay resident (32 KiB/partition total).
            # The query shard arrives pre-transposed (feature-major) so these
            # are plain contiguous row-block loads.
            xTs = []
            for p in range(NPAIR):
                xT = xp.tile([128, SL], bf16, tag=f"xT{p}")
                nc.sync.dma_start(out=xT[:], in_=xq[p * 128:(p + 1) * 128, :])
                xTs.append(xT)
            wq_sb = persist.tile([128, NPAIR, 128], bf16)
            nc.gpsimd.dma_start(out=wq_sb[:], in_=wq_bd.rearrange("p k m -> k p m"))
            wo_sb = persist.tile([128, NPAIR, E], bf16)
            nc.gpsimd.dma_start(
                out=wo_sb[:], in_=wo.rearrange("(k p) n -> p k n", p=128)
            )
            qfT = persist.tile([128, NPAIR, SL], bf16)
            ctxT = persist.tile([128, NPAIR, SL], bf16)

            # ---- phase K1: kf/v (s-major) + kv/z for every pair ----
            kv_in = dram.tile([128, NPAIR * 129], f32)
            z_in = dram.tile([128, NPAIR], f32)
            with (
                tc.tile_pool(name="ps_kvp", bufs=2, space="PSUM") as ps_kvp,
                tc.tile_pool(name="ps_kv", bufs=2, space="PSUM") as ps_kv,
            ):
                ones_sb = persist.tile([128, 1], bf16)
                nc.vector.memset(ones_sb[:], 1.0)
                for p in range(NPAIR):
                    xT = xTs[p]
                    kf = kvsb.tile([128, NCHUNK, 128], bf16, tag="kf")
                    vsb = kvsb.tile([128, NCHUNK, 128], bf16, tag="v")
                    kvacc = ps_kv.tile([128, 128], f32, tag="kvacc")
                    zacc = ps_kv.tile([128, 8], f32, tag="zacc")
                    for g in range(NCHUNK // 4):
                        kfps = ps_kvp.tile([128, 512], f32, tag="kfps")
                        vps = ps_kvp.tile([128, 512], f32, tag="vps")
                        for c4 in range(4):
                            i = g * 4 + c4
                            nc.tensor.matmul(
                                kfps[:, c4 * 128:(c4 + 1) * 128],
                                lhsT=xT[:, i * 128:(i + 1) * 128],
                                rhs=wkv_sb[:, p, 0:128],
                                start=True, stop=True,
                            )
                            nc.tensor.matmul(
                                vps[:, c4 * 128:(c4 + 1) * 128],
                                lhsT=xT[:, i * 128:(i + 1) * 128],
                                rhs=wkv_sb[:, p, 128:256],
                                start=True, stop=True,
                            )
                        cs = slice(g * 4, (g + 1) * 4)
                        kA = tmp.tile([128, 4, 128], bf16, tag="kA")
                        nc.scalar.activation(kA[:], kfps[:], Act.Identity, bias=1.0)
                        nc.vector.tensor_copy(vsb[:, cs, :], vps[:])
                        kM = tmp.tile([128, 4, 128], bf16, tag="kM")
                        nc.vector.tensor_scalar(
                            kM[:], kA[:], 1.0, -1.0, Alu.min, Alu.add
                        )
                        nc.scalar.activation(kM[:], kM[:], Act.Exp)
                        nc.vector.tensor_tensor(kf[:, cs, :], kA[:], kM[:], Alu.max)
                        for c4 in range(4):
                            i = g * 4 + c4
                            nc.tensor.matmul(
                                kvacc[:],
                                lhsT=kf[:, i, :], rhs=vsb[:, i, :],
                                start=(i == 0), stop=(i == NCHUNK - 1),
                            )
                            nc.tensor.matmul(
                                zacc[:, 0:1],
                                lhsT=kf[:, i, :], rhs=ones_sb[:],
                                start=(i == 0), stop=(i == NCHUNK - 1),
                            )
                    kvst = outp.tile([128, 129], f32, tag="kvst")
                    nc.vector.tensor_copy(kvst[:, 0:128], kvacc[:])
                    nc.vector.tensor_copy(kvst[:, 128:129], zacc[:, 0:1])
                    nc.sync.dma_start(
                        out=kv_in[:, p * 129:(p + 1) * 129], in_=kvst[:]
                    )
                    nc.sync.dma_start(
                        out=z_in[:, p:p + 1], in_=kvst[:, 128:129]
                    )

            # ---- phase R: z-only AllReduce first (small, unblocks den),
            # then the kv AllReduce; both overlap phase K2 below ----
            groups = [[0, 1], [2, 3], [4, 5], [6, 7]]
            z_ar = nc.dram_tensor("z_ar", [128, NPAIR], f32)
            nc.gpsimd.collective_compute(
                "AllReduce", Alu.add, replica_groups=groups,
                ins=[z_in[:]], outs=[z_ar[:]],
            )
            nc.gpsimd.collective_compute(
                "AllReduce", Alu.add, replica_groups=groups,
                ins=[kv_in[:]], outs=[kv_ar[:]],
            )
            z_rd = persist.tile([128, NPAIR], f32)
            nc.sync.dma_start(out=z_rd[:], in_=z_ar[:])
            # Per-pair zero-padded z columns for the den accumulation.
            zbd = persist.tile([128, NPAIR, H], bf16)
            nc.vector.memset(zbd[:], 0.0)
            for p in range(NPAIR):
                nc.vector.tensor_copy(
                    zbd[0:64, p, 2 * p:2 * p + 1], z_rd[0:64, p:p + 1]
                )
                nc.vector.tensor_copy(
                    zbd[64:128, p, 2 * p + 1:2 * p + 2], z_rd[64:128, p:p + 1]
                )

            # ---- phase K2: qf (feature-major) + den accumulation, both
            # overlapping the kv collective ----
            with (
                tc.tile_pool(name="ps_q", bufs=2, space="PSUM") as ps_q,
                tc.tile_pool(name="ps_den", bufs=1, space="PSUM") as ps_den,
            ):
                denps = ps_den.tile([16, SL], f32)
                for p in range(NPAIR):
                    xT = xTs[p]
                    for qc in range(2):
                        qs = slice(qc * 1024, (qc + 1) * 1024)
                        qps = ps_q.tile([128, 1024], f32, tag="qps")
                        nc.tensor.matmul(
                            qps[:, 0:512], lhsT=wq_sb[:, p, :],
                            rhs=xT[:, qc * 1024:qc * 1024 + 512],
                            start=True, stop=True,
                        )
                        nc.tensor.matmul(
                            qps[:, 512:1024], lhsT=wq_sb[:, p, :],
                            rhs=xT[:, qc * 1024 + 512:(qc + 1) * 1024],
                            start=True, stop=True,
                        )
                        qA = tmp.tile([128, 1024], bf16, tag="qA")
                        nc.scalar.activation(qA[:], qps[:], Act.Identity, bias=1.0)
                        qM = tmp.tile([128, 1024], bf16, tag="qM")
                        nc.vector.tensor_scalar(
                            qM[:], qA[:], 1.0, -1.0, Alu.min, Alu.add
                        )
                        nc.scalar.activation(qM[:], qM[:], Act.Exp)
                        nc.vector.tensor_tensor(qfT[:, p, qs], qA[:], qM[:], Alu.max)
                    for qc in range(NQC):
                        qs = slice(qc * 512, (qc + 1) * 512)
                        nc.tensor.matmul(
                            denps[:, qs], lhsT=zbd[:, p, :], rhs=qfT[:, p, qs],
                            start=(p == 0), stop=(p == NPAIR - 1),
                        )
                den_sb = persist.tile([16, SL], bf16)
                eps_sb = persist.tile([16, 1], f32)
                nc.vector.memset(eps_sb[:], EPS)
                nc.scalar.activation(
                    den_sb[:], denps[:], Act.Identity, bias=eps_sb[:]
                )
            kvrd = persist.tile([128, NPAIR, 129], f32)
            nc.sync.dma_start(
                out=kvrd[:], in_=kv_ar.rearrange("q (p c) -> q p c", c=129)
            )

            # ---- reciprocal of den on the Scalar engine LUT (one pass; its
            # ~1e-3 accuracy is invisible next to the bf16 quantization of the
            # reciprocal that follows) ----
            recip_bf = persist.tile([16, SL], bf16)
            eng = nc.scalar
            eng.add_instruction(
                mybir.InstActivation(
                    name=nc.get_next_instruction_name(),
                    func=Act.Reciprocal,
                    ins=[
                        eng.lower_ap(den_sb[:]),
                        mybir.ImmediateValue(dtype=f32, value=0.0),
                        mybir.ImmediateValue(dtype=f32, value=1.0),
                        mybir.ImmediateValue(dtype=f32, value=0.0),
                    ],
                    outs=[eng.lower_ap(recip_bf[:])],
                )
            )
            recip_dram = dram.tile([16, SL], bf16)
            nc.sync.dma_start(out=recip_dram[:], in_=recip_bf[:])

            # ---- phase N2: num + fused divide -> ctxT ----
            with tc.tile_pool(name="ps_num", bufs=3, space="PSUM") as ps_num:
                for p in range(NPAIR):
                    kvbd = tmp.tile([128, 128], bf16, tag="kvbd")
                    nc.vector.memset(kvbd[:], 0.0)
                    nc.vector.tensor_copy(kvbd[0:64, 0:64], kvrd[0:64, p, 0:64])
                    nc.vector.tensor_copy(
                        kvbd[64:128, 64:128], kvrd[64:128, p, 64:128]
                    )
                    for qc in range(2):
                        qs = slice(qc * 1024, (qc + 1) * 1024)
                        rbc = rbcp.tile([128, 1024], bf16, tag="rbc")
                        nc.sync.dma_start(
                            out=rbc[0:64, :],
                            in_=recip_dram[2 * p:2 * p + 1, qs].to_broadcast(
                                [64, 1024]
                            ),
                        )
                        nc.sync.dma_start(
                            out=rbc[64:128, :],
                            in_=recip_dram[2 * p + 1:2 * p + 2, qs].to_broadcast(
                                [64, 1024]
                            ),
                        )
                        nps = ps_num.tile([128, 1024], f32, tag="nps")
                        nc.tensor.matmul(
                            nps[:, 0:512], lhsT=kvbd[:],
                            rhs=qfT[:, p, qc * 1024:qc * 1024 + 512],
                            start=True, stop=True,
                        )
                        nc.tensor.matmul(
                            nps[:, 512:1024], lhsT=kvbd[:],
                            rhs=qfT[:, p, qc * 1024 + 512:(qc + 1) * 1024],
                            start=True, stop=True,
                        )
                        nc.vector.tensor_tensor(
                            ctxT[:, p, qs], nps[:], rbc[:], Alu.mult
                        )

            # ---- phase O: output projection (sequence-major out) ----
            with tc.tile_pool(name="ps_o", bufs=2, space="PSUM") as ps_o:
                for si in range(NSO):
                    ss = slice(si * 128, (si + 1) * 128)
                    ops = ps_o.tile([128, E], f32, tag="ops")
                    for k in range(NPAIR):
                        nc.tensor.matmul(
                            ops[:, 0:512], lhsT=ctxT[:, k, ss],
                            rhs=wo_sb[:, k, 0:512],
                            start=(k == 0), stop=(k == NPAIR - 1),
                        )
                        nc.tensor.matmul(
                            ops[:, 512:E], lhsT=ctxT[:, k, ss],
                            rhs=wo_sb[:, k, 512:E],
                            start=(k == 0), stop=(k == NPAIR - 1),
                        )
                    for oc in range(2):
                        ysb = outp.tile([128, 512], f32, tag="ysb")
                        if oc == 0:
                            nc.vector.tensor_copy(ysb[:], ops[:, 0:512])
                        else:
                            nc.scalar.copy(ysb[:], ops[:, 512:1024])
                        nc.sync.dma_start(
                            out=y[ss, oc * 512:(oc + 1) * 512], in_=ysb[:]
                        )

    nc.compile()
    return nc


def _get_program():
    if "nc" not in _CACHE:
        _CACHE["nc"] = _build_program()
    return _CACHE["nc"]


def _host_prep(query, Wq, Wk, Wv, Wo):
    bf16 = ml_dtypes.bfloat16
    q_bf = np.ascontiguousarray(query.astype(bf16))
    wq_bd = np.zeros((NPAIR, 128, 128), dtype=bf16)
    wkv_bd = np.zeros((NPAIR, 128, 256), dtype=bf16)
    for p in range(NPAIR):
        wq_bd[p, 0:64, 0:64] = Wq[2 * p]
        wq_bd[p, 64:128, 64:128] = Wq[2 * p + 1]
        wkv_bd[p, 0:64, 0:64] = Wk[2 * p]
        wkv_bd[p, 64:128, 64:128] = Wk[2 * p + 1]
        wkv_bd[p, 0:64, 128:192] = Wv[2 * p]
        wkv_bd[p, 64:128, 192:256] = Wv[2 * p + 1]
    wo_bf = np.ascontiguousarray(Wo.astype(bf16))
    in_maps = []
    for c in range(N_CORES):
        b, j = divmod(c, 2)
        in_maps.append({
            "xqT": np.ascontiguousarray(q_bf[b, j * SL:(j + 1) * SL, :].T),
            "wq_bd": wq_bd,
            "wkv_bd": wkv_bd,
            "wo": wo_bf,
        })
    return in_maps


def kernel(query, Wq, Wk, Wv, Wo):
    from concourse.bass_utils import run_bass_kernel_spmd

    nc = _get_program()
    in_maps = _host_prep(query, Wq, Wk, Wv, Wo)
    res = run_bass_kernel_spmd(nc, in_maps, list(range(N_CORES)))
    out = np.empty((B, S, E), dtype=np.float32)
    for c in range(N_CORES):
        b, j = divmod(c, 2)
        out[b, j * SL:(j + 1) * SL, :] = res.results[c]["y"]
    return out
